# revision 6
# baseline (speedup 1.0000x reference)
"""MultiHeadAttention Trainium2 kernel (8 NeuronCores, SPMD) — v2.

Sharding: core c = (batch b=c//4, head-group g=c%4); each core owns 4 of 16
heads for one batch element. Wq/Wk/Wv split by output features (tensor
parallel on heads), Wo by input features (row parallel); the 4 partial
[S, D] outputs per batch are summed on the host.

v2 layout: ctx accumulates in [q, feat] orientation (e-tiles as the matmul
stationary, N=65 streams) so softmax denominators land as a per-partition
column and normalization is a tensor_scalar_mul — no partition broadcasts.
Normalized ctx is PE-transposed back to [feat, q] for the out-projection.
The kt loop is software-pipelined (scores/exp of kt+1 issue before ctx of
kt) so the PE never head-of-line blocks on the ACT engine's exp.
"""

import numpy as np

B, S, D = 2, 2048, 1024
H, DK = 16, 64
HG = 4                 # heads per core
FC = HG * DK           # 256 features per core
NCORES = 8
P = 128
KSUB = D // P          # 8 contraction subtiles for projections
FT = FC // P           # 2 feature tiles (= head pairs)
NKT = S // P           # 16 key-position tiles
QC = 512               # q-chunk size
NQC = S // QC          # 4
NQT = QC // P          # 4 q-subtiles per chunk
SCHUNK = 512           # s-chunk for streaming projections
NSC = S // SCHUNK      # 4

_PROGRAM = None        # cached Bass program - build once per process


def _build_program():
    from contextlib import ExitStack

    import concourse.bass as bass
    import concourse.mybir as mybir
    import concourse.tile as tile
    from concourse import bacc, masks

    f32 = mybir.dt.float32
    bf16 = mybir.dt.bfloat16
    EXP = mybir.ActivationFunctionType.Exp

    nc = bacc.Bacc("TRN2", target_bir_lowering=False, debug=False)

    qT = nc.dram_tensor("qT", [D, S], bf16, kind="ExternalInput")
    kT = nc.dram_tensor("kT", [D, S], bf16, kind="ExternalInput")
    vT = nc.dram_tensor("vT", [D, S], bf16, kind="ExternalInput")
    wqT = nc.dram_tensor("wqT", [D, FC], bf16, kind="ExternalInput")
    wkT = nc.dram_tensor("wkT", [D, FC], bf16, kind="ExternalInput")
    wvT = nc.dram_tensor("wvT", [D, FC], bf16, kind="ExternalInput")
    woT = nc.dram_tensor("woT", [FC, D], bf16, kind="ExternalInput")
    bq = nc.dram_tensor("bq", [FC], f32, kind="ExternalInput")
    bk = nc.dram_tensor("bk", [FC], f32, kind="ExternalInput")
    out = nc.dram_tensor("out", [S, D], f32, kind="ExternalOutput")

    with tile.TileContext(nc) as tc, ExitStack() as ctx, nc.allow_low_precision(
        reason="bf16 matmul operands are intentional"
    ):
        weights = ctx.enter_context(tc.tile_pool(name="weights", bufs=1))
        instream = ctx.enter_context(tc.tile_pool(name="instream", bufs=12))
        persist = ctx.enter_context(tc.tile_pool(name="persist", bufs=1))
        exps = ctx.enter_context(tc.tile_pool(name="exps", bufs=8))
        ctn_p = ctx.enter_context(tc.tile_pool(name="ctn_p", bufs=8))
        ctsb_p = ctx.enter_context(tc.tile_pool(name="ctsb_p", bufs=2))
        outsb = ctx.enter_context(tc.tile_pool(name="outsb", bufs=2))
        small = ctx.enter_context(tc.tile_pool(name="small", bufs=8))
        # PSUM: tag "sc" [128,1024]f32 = 2 banks x2 bufs; the per-qc packed
        # ct^T ([128,1024]bf16, 1 bank) shares the tag. tag "acc"
        # [128,512]f32 = 1 bank x4: proj accumulators, per-qt ctx
        # accumulators, out-proj tiles.
        ps = ctx.enter_context(tc.tile_pool(name="ps", bufs=2, space="PSUM"))

        # ---- persistent weights (K/Q first — they gate the first scores) ----
        wk_sb = weights.tile([P, KSUB, FC], bf16, tag="wk")
        nc.sync.dma_start(wk_sb, wkT[:, :].rearrange("(o p) f -> p o f", p=P))
        wq_sb = weights.tile([P, KSUB, FC], bf16, tag="wq")
        nc.sync.dma_start(wq_sb, wqT[:, :].rearrange("(o p) f -> p o f", p=P))
        bq_sb = weights.tile([P, FT], f32, tag="bq")
        nc.sync.dma_start(bq_sb, bq[:].rearrange("(t p) -> p t", p=P))
        bk_sb = weights.tile([P, FT], f32, tag="bk")
        nc.sync.dma_start(bk_sb, bk[:].rearrange("(t p) -> p t", p=P))
        ident = weights.tile([P, P], bf16, tag="ident")
        masks.make_identity(nc, ident)
        # warm up the PE p-state while the first input DMAs are in flight
        # (independent column slices — no WAW chain, streams back-to-back)
        warm = ps.tile([P, 8 * P], bf16, tag="sc", name="warm")
        for i in range(52):
            nc.tensor.matmul(
                warm[:, (i % 8) * P:(i % 8) * P + P],
                lhsT=ident, rhs=ident, is_transpose=True,
                start=True, stop=True,
            )

        # ---- persistent activations ----
        QT = persist.tile([P, FT, S], bf16, tag="QT")   # [feat, seq]
        KT = persist.tile([P, FT, S], bf16, tag="KT")   # [feat, seq]
        V = persist.tile([P, NKT, HG, 66], bf16, tag="V")  # [seq, h, dk+ones]
        nc.vector.memset(V[:, :, :, 64:65], 1.0)

        # ---- input prefetch (all 12 chunk DMAs up front; K/V first) ----
        qTr = qT[:, :].rearrange("(o p) s -> p o s", p=P)
        kTr = kT[:, :].rearrange("(o p) s -> p o s", p=P)
        vTr = vT[:, :].rearrange("(o p) s -> p o s", p=P)
        kcs, vcs, qcs = [None] * NSC, [None] * NSC, [None] * NSC

        def dma_in(lst, src, c, nm):
            t = instream.tile([P, KSUB, SCHUNK], bf16, tag="instream",
                              name=f"{nm}{c}")
            nc.sync.dma_start(t, src[:, :, c * SCHUNK:(c + 1) * SCHUNK])
            lst[c] = t

        dma_in(kcs, kTr, 0, "kc")
        dma_in(qcs, qTr, 0, "qc")
        wv_sb = weights.tile([P, KSUB, FC], bf16, tag="wv")
        nc.sync.dma_start(wv_sb, wvT[:, :].rearrange("(o p) f -> p o f", p=P))
        dma_in(vcs, vTr, 0, "vc")
        wo_sb = weights.tile([P, FT, D], bf16, tag="wo")
        nc.sync.dma_start(wo_sb, woT[:, :].rearrange("(t p) j -> p t j", p=P))
        for c in range(1, NSC):
            dma_in(kcs, kTr, c, "kc")
            dma_in(vcs, vTr, c, "vc")
        for c in range(1, NSC):
            dma_in(qcs, qTr, c, "qc")

        # ---- projection group emitters: transients rotate through the
        # "sc" PSUM slots, so they can interleave with attention without
        # deadlocking against the long-lived ctx accumulators. ----
        def emit_kproj_g(c, ft):
            sl = slice(c * SCHUNK, (c + 1) * SCHUNK)
            pk = ps.tile([P, SCHUNK], f32, tag="sc", name=f"pk{c}_{ft}")
            for ks in range(KSUB):
                nc.tensor.matmul(
                    pk,
                    lhsT=wk_sb[:, ks, ft * P:(ft + 1) * P],
                    rhs=kcs[c][:, ks, :],
                    start=(ks == 0),
                    stop=(ks == KSUB - 1),
                )
            nc.vector.tensor_scalar_add(KT[:, ft, sl], pk, bk_sb[:, ft:ft + 1])

        def emit_vproj_g(c, st):
            pv = ps.tile([P, FC], f32, tag="sc", name=f"pv{c}_{st}")
            for ks in range(KSUB):
                nc.tensor.matmul(
                    pv,
                    lhsT=vcs[c][:, ks, st * P:(st + 1) * P],
                    rhs=wv_sb[:, ks, :],
                    start=(ks == 0),
                    stop=(ks == KSUB - 1),
                )
            kt_i = c * (SCHUNK // P) + st
            nc.vector.tensor_copy(
                out=V[:, kt_i, :, 0:DK],
                in_=pv[:, :].rearrange("p (h d) -> p h d", h=HG),
            )

        def emit_qproj_g(c, ft):
            sl = slice(c * SCHUNK, (c + 1) * SCHUNK)
            pq = ps.tile([P, SCHUNK], f32, tag="sc", name=f"pq{c}_{ft}")
            for ks in range(KSUB):
                nc.tensor.matmul(
                    pq,
                    lhsT=wq_sb[:, ks, ft * P:(ft + 1) * P],
                    rhs=qcs[c][:, ks, :],
                    start=(ks == 0),
                    stop=(ks == KSUB - 1),
                )
            nc.vector.tensor_scalar_add(QT[:, ft, sl], pq, bq_sb[:, ft:ft + 1])

        # ---- upfront: K/Q chunk 0 only (they gate the first scores);
        # V chunk 0 and everything else streams into the attention loop ----
        for ft in range(FT):
            emit_kproj_g(0, ft)
        for ft in range(FT):
            emit_qproj_g(0, ft)

        # ---- attention, software-pipelined over kt ----
        def emit_scores_exp(qc, kt):
            qsl = slice(qc * QC, (qc + 1) * QC)
            ksl = slice(kt * P, (kt + 1) * P)
            es = []
            for ft in range(FT):
                sc = ps.tile([P, 2 * QC], f32, tag="sc",
                             name=f"sc{qc}_{kt}_{ft}")
                nc.tensor.matmul(
                    sc[:, 0:QC],
                    lhsT=KT[0:64, ft, ksl],
                    rhs=QT[0:64, ft, qsl],
                    start=True, stop=True,
                    tile_position=(0, 0),
                )
                nc.tensor.matmul(
                    sc[:, QC:2 * QC],
                    lhsT=KT[64:128, ft, ksl],
                    rhs=QT[64:128, ft, qsl],
                    start=True, stop=True,
                    tile_position=(64, 0),
                )
                e = exps.tile([P, 2 * QC], bf16, tag="exps",
                              name=f"e{qc}_{kt}_{ft}")
                nc.scalar.activation(e, sc, EXP)
                es.append(e)
            return es

        def emit_ctx(ctxu, kt, es):
            # PSUM zero regions are whole 2KB banks: head 0's start marks
            # the bank pending-zero, so heads 1-3's first writes overwrite
            # (not accumulate) without their own start; only the final
            # head/kt closes the group.
            first, last = kt == 0, kt == NKT - 1
            for qt in range(NQT):
                for h in range(HG):
                    ft, half = divmod(h, 2)
                    nc.tensor.matmul(
                        ctxu[qt][:, h * 65:h * 65 + 65],
                        lhsT=es[ft][:, half * QC + qt * P:half * QC + (qt + 1) * P],
                        rhs=V[:, kt, h, 0:65],
                        start=first and h == 0,
                        stop=last and h == HG - 1,
                    )

        def emit_outproj_st(qc_, ctsb_, st, tail=False):
            s0 = qc_ * QC + st * P
            osb = outsb.tile([P, D], f32, tag="osb",
                             name=f"osb{qc_}_{st}")
            for jc in range(D // 512):
                ops = ps.tile([P, 512], f32, tag="sc",
                              name=f"ops{qc_}_{st}_{jc}")
                for ft in range(FT):
                    nc.tensor.matmul(
                        ops,
                        lhsT=ctsb_[:, ft, st * P:(st + 1) * P],
                        rhs=wo_sb[:, ft, jc * 512:(jc + 1) * 512],
                        start=(ft == 0),
                        stop=(ft == FT - 1),
                    )
                dst = osb[:, jc * 512:(jc + 1) * 512]
                if tail and jc == 0:
                    # ACT is idle after the last exp — keep the DVE free
                    # for the next wave's normalization (GPSIMD cannot
                    # read PSUM, so the choices are ACT and DVE)
                    nc.scalar.copy(dst, ops)
                else:
                    nc.vector.tensor_copy(out=dst, in_=ops)
                if tail:
                    nc.sync.dma_start(
                        out[s0:s0 + P, jc * 512:(jc + 1) * 512], dst
                    )
            if not tail:
                nc.sync.dma_start(out[s0:s0 + P, :], osb)

        # epilogue for one 128-q block: normalize (tensor_scalar by the
        # 1/denom column), PE-transpose back to [feat, q], pack into the
        # out-proj lhsT. ctT rides the freed "acc" slots.
        ctxus, ctsbs, ctTs = {}, {}, {}

        def alloc_epilogue(qc_):
            # Must be called BEFORE the next chunk's ctxu allocations so
            # ctT lands on the slot freed by this chunk's own qt0 — not on
            # a slot whose release depends on the next chunk completing.
            ctsbs[qc_] = ctsb_p.tile([P, FT, QC], bf16, tag="ctsb",
                                     name=f"ctsb{qc_}")
            ctTs[qc_] = ps.tile([P, FT * QC], bf16, tag="acc", bufs=4,
                                name=f"ctT{qc_}")

        ctns = {}

        def emit_norm_qt(qc_, qt):
            cx = ctxus[qc_][qt][:, :].rearrange("p (h c) -> p h c", h=HG)
            recip = small.tile([P, HG], f32, tag="recip",
                               name=f"recip{qc_}_{qt}")
            nc.vector.reciprocal(recip, cx[:, :, 64])
            ctn = ctn_p.tile([P, HG, DK], bf16, tag="ctn",
                             name=f"ctn{qc_}_{qt}")
            for h in range(HG):
                nc.vector.tensor_scalar_mul(
                    ctn[:, h, :], cx[:, h, 0:64], recip[:, h:h + 1]
                )
            ctns[(qc_, qt)] = ctn

        def emit_pack_qt(qc_, qt, tail=False):
            ctsb, ctT = ctsbs[qc_], ctTs[qc_]
            ctn = ctns[(qc_, qt)]
            for h in range(HG):
                ft, half = divmod(h, 2)
                nc.tensor.matmul(
                    ctT[half * 64:(half + 1) * 64,
                        ft * QC + qt * P:ft * QC + (qt + 1) * P],
                    lhsT=ctn[:, h, :],
                    rhs=ident,
                    is_transpose=True,
                    start=True, stop=True,
                )
            ctTr = ctT[:, :].rearrange("p (t q) -> p t q", t=FT)
            if tail:
                # tail: ACT is idle after the last exp — use it for the
                # pack copy so the DVE chain isn't the tail critical path
                nc.scalar.copy(
                    ctsb[:, :, qt * P:(qt + 1) * P],
                    ctTr[:, :, qt * P:(qt + 1) * P],
                )
            else:
                nc.vector.tensor_copy(
                    out=ctsb[:, :, qt * P:(qt + 1) * P],
                    in_=ctTr[:, :, qt * P:(qt + 1) * P],
                )

        def emit_epilogue_qt(qc_, qt):
            emit_norm_qt(qc_, qt)
            emit_pack_qt(qc_, qt)



        # per-kt-slot PE filler work, emitted inside the attention loop.
        # qc0 streams V chunk 0 then the K/V projections for chunks 1-3
        # (paced so chunk c lands before scores need it at kt=4c). Each
        # qc>0 runs the previous chunk's epilogue (slots 0-1) and
        # out-projection (spread one st per slot), and projects the next
        # q-chunk late in the loop.
        kv_groups = [("v", 0, st) for st in range(SCHUNK // P)]
        for c in range(1, NSC):
            kv_groups += [("k", c, ft) for ft in range(FT)]
            kv_groups += [("v", c, st) for st in range(SCHUNK // P)]
        inserts = {qc: {kt: [] for kt in range(NKT)} for qc in range(NQC)}
        for i, g in enumerate(kv_groups[:4]):   # V0: slots 0-1
            inserts[0][i // 2].append(g)
        for i, g in enumerate(kv_groups[4:]):   # 18 groups, pace 1.5/slot
            inserts[0][2 + (i * 2) // 3].append(g)
        for qc in range(NQC - 1):
            inserts[qc][11].append(("q", qc + 1, 0))
            inserts[qc][13].append(("q", qc + 1, 1))
        for qc in range(1, NQC):
            for qt in range(NQT):
                inserts[qc][qt // 2].append(("e", qc - 1, qt))
            for st in range(NQT):
                inserts[qc][3 + 2 * st].append(("o", qc - 1, st))

        for qc in range(NQC):
            if qc > 0:
                alloc_epilogue(qc - 1)
            ctxus[qc] = [
                ps.tile([P, HG * 65], f32, tag="acc", bufs=4,
                        name=f"ctxu{qc}_{qt}")
                for qt in range(NQT)
            ]
            # ctx lags scores by 2 kt so the PE's in-order stream never
            # puts ctx(kt) — which waits on both of kt's exps — ahead of
            # scores(kt+1) that feed the ACT engine.
            pend = []
            for kt in range(NKT):
                es = emit_scores_exp(qc, kt)
                pend.append((kt, es))
                for item in inserts[qc][kt]:
                    kind, a, b = item[0], item[1], item[2]
                    if kind == "k":
                        emit_kproj_g(a, b)
                    elif kind == "v":
                        emit_vproj_g(a, b)
                    elif kind == "q":
                        emit_qproj_g(a, b)
                    elif kind == "e":
                        emit_epilogue_qt(a, b)
                    else:
                        emit_outproj_st(a, ctsbs[a], b)
                if len(pend) > 2:
                    k0, e0 = pend.pop(0)
                    emit_ctx(ctxus[qc], k0, e0)
            for k0, e0 in pend:
                emit_ctx(ctxus[qc], k0, e0)

        # tail: last q-chunk's epilogue + direct-DMA out-projection,
        # pipelined per 128-q block.
        last = NQC - 1
        alloc_epilogue(last)
        for qt in range(NQT):
            emit_norm_qt(last, qt)
        for qt in range(NQT):
            emit_pack_qt(last, qt, tail=True)
            emit_outproj_st(last, ctsbs[last], qt, tail=True)

    nc.compile()
    return nc


def _get_program():
    global _PROGRAM
    if _PROGRAM is None:
        _PROGRAM = _build_program()
    return _PROGRAM


def _host_shards(q, k, v, Wq, bq, Wk, bk, Wv, bv, Wo, bo):
    """Build the 8 per-core input dicts (host-side transposes/slices)."""
    import ml_dtypes

    b16 = ml_dtypes.bfloat16
    scale = 1.0 / np.sqrt(np.float32(DK))
    qT = [np.ascontiguousarray(q[b].T).astype(b16) for b in range(B)]
    kT = [np.ascontiguousarray(k[b].T).astype(b16) for b in range(B)]
    vT = [np.ascontiguousarray(v[b].T).astype(b16) for b in range(B)]
    in_maps = []
    for c in range(NCORES):
        b, g = divmod(c, NCORES // B)
        fsl = slice(g * FC, (g + 1) * FC)
        in_maps.append({
            "qT": qT[b],
            "kT": kT[b],
            "vT": vT[b],
            "wqT": np.ascontiguousarray(Wq[fsl, :].T * scale).astype(b16),
            "wkT": np.ascontiguousarray(Wk[fsl, :].T).astype(b16),
            "wvT": np.ascontiguousarray(Wv[fsl, :].T).astype(b16),
            "woT": np.ascontiguousarray(Wo[:, fsl].T).astype(b16),
            "bq": np.ascontiguousarray(bq[fsl] * scale),
            "bk": np.ascontiguousarray(bk[fsl]),
        })
    return in_maps


def kernel(q, k, v, mask, Wq, bq, Wk, bk, Wv, bv, Wo, bo):
    q = np.asarray(q, dtype=np.float32)
    k = np.asarray(k, dtype=np.float32)
    v = np.asarray(v, dtype=np.float32)
    mask = np.asarray(mask)
    Wq = np.asarray(Wq, dtype=np.float32)
    bq = np.asarray(bq, dtype=np.float32)
    Wk = np.asarray(Wk, dtype=np.float32)
    bk = np.asarray(bk, dtype=np.float32)
    Wv = np.asarray(Wv, dtype=np.float32)
    bv = np.asarray(bv, dtype=np.float32)
    Wo = np.asarray(Wo, dtype=np.float32)
    bo = np.asarray(bo, dtype=np.float32)

    if not np.all(mask != 0):
        # Unmasked-path kernel; fall back to exact host computation if a
        # nontrivial mask ever shows up (spec fills the mask with ones).
        return _host_reference(q, k, v, mask, Wq, bq, Wk, bk, Wv, bv, Wo, bo)

    from concourse.bass_utils import run_bass_kernel_spmd

    nc = _get_program()
    in_maps = _host_shards(q, k, v, Wq, bq, Wk, bk, Wv, bv, Wo, bo)
    res = run_bass_kernel_spmd(nc, in_maps, core_ids=list(range(NCORES)))

    # host reduction: sum the 4 row-parallel Wo partials per batch,
    # then add the exact bv/bo correction (softmax rows sum to 1).
    const = bv @ Wo.T + bo
    out = np.empty((B, S, D), np.float32)
    gpb = NCORES // B
    for b in range(B):
        acc = res.results[b * gpb]["out"].astype(np.float32)
        for g in range(1, gpb):
            acc = acc + res.results[b * gpb + g]["out"]
        out[b] = acc + const[None, :]
    return out


def _host_reference(q, k, v, mask, Wq, bq, Wk, bk, Wv, bv, Wo, bo):
    def split_heads(x):
        b, s, _ = x.shape
        return x.reshape(b, s, H, DK).transpose(0, 2, 1, 3)

    query = split_heads(q @ Wq.T + bq)
    key_ = split_heads(k @ Wk.T + bk)
    value = split_heads(v @ Wv.T + bv)
    scores = np.einsum("bhqd,bhkd->bhqk", query, key_) / np.sqrt(np.float32(DK))
    scores = np.where(mask == 0, np.float32(-1e9), scores).astype(np.float32)
    scores -= scores.max(axis=-1, keepdims=True)
    e = np.exp(scores)
    attn = e / e.sum(axis=-1, keepdims=True)
    ctx = np.einsum("bhqk,bhkd->bhqd", attn, value)
    ctx = ctx.transpose(0, 2, 1, 3).reshape(q.shape[0], -1, D)
    return (ctx @ Wo.T + bo).astype(np.float32)


# revision 7
# speedup vs baseline: 1.0479x; 1.0479x over previous
"""MultiHeadAttention Trainium2 kernel (8 NeuronCores, SPMD) — v2.

Sharding: core c = (batch b=c//4, head-group g=c%4); each core owns 4 of 16
heads for one batch element. Wq/Wk/Wv split by output features (tensor
parallel on heads), Wo by input features (row parallel); the 4 partial
[S, D] outputs per batch are summed on the host.

v2 layout: ctx accumulates in [q, feat] orientation (e-tiles as the matmul
stationary, N=65 streams) so softmax denominators land as a per-partition
column and normalization is a tensor_scalar_mul — no partition broadcasts.
Normalized ctx is PE-transposed back to [feat, q] for the out-projection.
The kt loop is software-pipelined (scores/exp of kt+1 issue before ctx of
kt) so the PE never head-of-line blocks on the ACT engine's exp.
"""

import numpy as np

B, S, D = 2, 2048, 1024
H, DK = 16, 64
HG = 4                 # heads per core
FC = HG * DK           # 256 features per core
NCORES = 8
P = 128
KSUB = D // P          # 8 contraction subtiles for projections
FT = FC // P           # 2 feature tiles (= head pairs)
NKT = S // P           # 16 key-position tiles
QC = 512               # q-chunk size
NQC = S // QC          # 4
NQT = QC // P          # 4 q-subtiles per chunk
SCHUNK = 512           # s-chunk for streaming projections
NSC = S // SCHUNK      # 4

_PROGRAM = None        # cached Bass program - build once per process


def _build_program():
    from contextlib import ExitStack

    import concourse.bass as bass
    import concourse.mybir as mybir
    import concourse.tile as tile
    from concourse import bacc, masks

    f32 = mybir.dt.float32
    bf16 = mybir.dt.bfloat16
    EXP = mybir.ActivationFunctionType.Exp

    nc = bacc.Bacc("TRN2", target_bir_lowering=False, debug=False)

    qT = nc.dram_tensor("qT", [D, S], bf16, kind="ExternalInput")
    kT = nc.dram_tensor("kT", [D, S], bf16, kind="ExternalInput")
    vT = nc.dram_tensor("vT", [D, S], bf16, kind="ExternalInput")
    wqT = nc.dram_tensor("wqT", [D, FC], bf16, kind="ExternalInput")
    wkT = nc.dram_tensor("wkT", [D, FC], bf16, kind="ExternalInput")
    wvT = nc.dram_tensor("wvT", [D, FC], bf16, kind="ExternalInput")
    woT = nc.dram_tensor("woT", [FC, D], bf16, kind="ExternalInput")
    bq = nc.dram_tensor("bq", [FC], f32, kind="ExternalInput")
    bk = nc.dram_tensor("bk", [FC], f32, kind="ExternalInput")
    out = nc.dram_tensor("out", [S, D], f32, kind="ExternalOutput")

    with tile.TileContext(nc) as tc, ExitStack() as ctx, nc.allow_low_precision(
        reason="bf16 matmul operands are intentional"
    ):
        weights = ctx.enter_context(tc.tile_pool(name="weights", bufs=1))
        instream = ctx.enter_context(tc.tile_pool(name="instream", bufs=12))
        persist = ctx.enter_context(tc.tile_pool(name="persist", bufs=1))
        exps = ctx.enter_context(tc.tile_pool(name="exps", bufs=8))
        ctn_p = ctx.enter_context(tc.tile_pool(name="ctn_p", bufs=8))
        ctsb_p = ctx.enter_context(tc.tile_pool(name="ctsb_p", bufs=2))
        outsb = ctx.enter_context(tc.tile_pool(name="outsb", bufs=2))
        small = ctx.enter_context(tc.tile_pool(name="small", bufs=8))
        # PSUM: tag "sc" [128,1024]f32 = 2 banks x2 bufs; the per-qc packed
        # ct^T ([128,1024]bf16, 1 bank) shares the tag. tag "acc"
        # [128,512]f32 = 1 bank x4: proj accumulators, per-qt ctx
        # accumulators, out-proj tiles.
        ps = ctx.enter_context(tc.tile_pool(name="ps", bufs=2, space="PSUM"))

        # ---- persistent weights (K/Q first — they gate the first scores) ----
        wk_sb = weights.tile([P, KSUB, FC], bf16, tag="wk")
        nc.sync.dma_start(wk_sb, wkT[:, :].rearrange("(o p) f -> p o f", p=P))
        wq_sb = weights.tile([P, KSUB, FC], bf16, tag="wq")
        nc.sync.dma_start(wq_sb, wqT[:, :].rearrange("(o p) f -> p o f", p=P))
        bq_sb = weights.tile([P, FT], f32, tag="bq")
        nc.sync.dma_start(bq_sb, bq[:].rearrange("(t p) -> p t", p=P))
        bk_sb = weights.tile([P, FT], f32, tag="bk")
        nc.sync.dma_start(bk_sb, bk[:].rearrange("(t p) -> p t", p=P))
        ident = weights.tile([P, P], bf16, tag="ident")
        masks.make_identity(nc, ident)
        # warm up the PE p-state while the first input DMAs are in flight
        # (independent column slices — no WAW chain, streams back-to-back)
        warm = ps.tile([P, 8 * P], bf16, tag="sc", name="warm")
        for i in range(52):
            nc.tensor.matmul(
                warm[:, (i % 8) * P:(i % 8) * P + P],
                lhsT=ident, rhs=ident, is_transpose=True,
                start=True, stop=True,
            )

        # ---- persistent activations ----
        QT = persist.tile([P, FT, S], bf16, tag="QT")   # [feat, seq]
        KT = persist.tile([P, FT, S], bf16, tag="KT")   # [feat, seq]
        V = persist.tile([P, NKT, HG, 66], bf16, tag="V")  # [seq, h, dk+ones]
        nc.vector.memset(V[:, :, :, 64:65], 1.0)

        # ---- input prefetch (all 12 chunk DMAs up front; K/V first) ----
        qTr = qT[:, :].rearrange("(o p) s -> p o s", p=P)
        kTr = kT[:, :].rearrange("(o p) s -> p o s", p=P)
        vTr = vT[:, :].rearrange("(o p) s -> p o s", p=P)
        kcs, vcs, qcs = [None] * NSC, [None] * NSC, [None] * NSC

        def dma_in(lst, src, c, nm):
            t = instream.tile([P, KSUB, SCHUNK], bf16, tag="instream",
                              name=f"{nm}{c}")
            nc.sync.dma_start(t, src[:, :, c * SCHUNK:(c + 1) * SCHUNK])
            lst[c] = t

        dma_in(kcs, kTr, 0, "kc")
        # q chunk 0 lands in two halves so its projection can start early
        qc0_t = instream.tile([P, KSUB, SCHUNK], bf16, tag="instream",
                              name="qc0")
        nc.sync.dma_start(qc0_t[:, :, 0:SCHUNK // 2],
                          qTr[:, :, 0:SCHUNK // 2])
        nc.sync.dma_start(qc0_t[:, :, SCHUNK // 2:SCHUNK],
                          qTr[:, :, SCHUNK // 2:SCHUNK])
        qcs[0] = qc0_t
        wv_sb = weights.tile([P, KSUB, FC], bf16, tag="wv")
        nc.sync.dma_start(wv_sb, wvT[:, :].rearrange("(o p) f -> p o f", p=P))
        dma_in(vcs, vTr, 0, "vc")
        wo_sb = weights.tile([P, FT, D], bf16, tag="wo")
        nc.sync.dma_start(wo_sb, woT[:, :].rearrange("(t p) j -> p t j", p=P))
        for c in range(1, NSC):
            dma_in(kcs, kTr, c, "kc")
            dma_in(vcs, vTr, c, "vc")
        for c in range(1, NSC):
            dma_in(qcs, qTr, c, "qc")

        # ---- projection group emitters: transients rotate through the
        # "sc" PSUM slots, so they can interleave with attention without
        # deadlocking against the long-lived ctx accumulators. ----
        def emit_kproj_g(c, ft):
            sl = slice(c * SCHUNK, (c + 1) * SCHUNK)
            pk = ps.tile([P, SCHUNK], f32, tag="sc", name=f"pk{c}_{ft}")
            for ks in range(KSUB):
                nc.tensor.matmul(
                    pk,
                    lhsT=wk_sb[:, ks, ft * P:(ft + 1) * P],
                    rhs=kcs[c][:, ks, :],
                    start=(ks == 0),
                    stop=(ks == KSUB - 1),
                )
            nc.vector.tensor_scalar_add(KT[:, ft, sl], pk, bk_sb[:, ft:ft + 1])

        def emit_vproj_g(c, st):
            pv = ps.tile([P, FC], f32, tag="sc", name=f"pv{c}_{st}")
            for ks in range(KSUB):
                nc.tensor.matmul(
                    pv,
                    lhsT=vcs[c][:, ks, st * P:(st + 1) * P],
                    rhs=wv_sb[:, ks, :],
                    start=(ks == 0),
                    stop=(ks == KSUB - 1),
                )
            kt_i = c * (SCHUNK // P) + st
            nc.vector.tensor_copy(
                out=V[:, kt_i, :, 0:DK],
                in_=pv[:, :].rearrange("p (h d) -> p h d", h=HG),
            )

        def emit_qproj_g(c, ft):
            sl = slice(c * SCHUNK, (c + 1) * SCHUNK)
            pq = ps.tile([P, SCHUNK], f32, tag="sc", name=f"pq{c}_{ft}")
            for ks in range(KSUB):
                nc.tensor.matmul(
                    pq,
                    lhsT=wq_sb[:, ks, ft * P:(ft + 1) * P],
                    rhs=qcs[c][:, ks, :],
                    start=(ks == 0),
                    stop=(ks == KSUB - 1),
                )
            nc.vector.tensor_scalar_add(QT[:, ft, sl], pq, bq_sb[:, ft:ft + 1])

        # ---- upfront: K/Q chunk 0 only (they gate the first scores);
        # V chunk 0 and everything else streams into the attention loop.
        # Q0 is projected in 256-column halves to chase its split DMA. ----
        for ft in range(FT):
            emit_kproj_g(0, ft)
        half = SCHUNK // 2
        for hi in range(2):
            for ft in range(FT):
                pq0 = ps.tile([P, half], f32, tag="sc", name=f"pq0_{hi}_{ft}")
                for ks in range(KSUB):
                    nc.tensor.matmul(
                        pq0,
                        lhsT=wq_sb[:, ks, ft * P:(ft + 1) * P],
                        rhs=qcs[0][:, ks, hi * half:(hi + 1) * half],
                        start=(ks == 0),
                        stop=(ks == KSUB - 1),
                    )
                nc.vector.tensor_scalar_add(
                    QT[:, ft, hi * half:(hi + 1) * half], pq0,
                    bq_sb[:, ft:ft + 1],
                )

        # ---- attention, software-pipelined over kt ----
        def emit_scores_exp(qc, kt):
            qsl = slice(qc * QC, (qc + 1) * QC)
            ksl = slice(kt * P, (kt + 1) * P)
            es = []
            for ft in range(FT):
                sc = ps.tile([P, 2 * QC], f32, tag="sc",
                             name=f"sc{qc}_{kt}_{ft}")
                nc.tensor.matmul(
                    sc[:, 0:QC],
                    lhsT=KT[0:64, ft, ksl],
                    rhs=QT[0:64, ft, qsl],
                    start=True, stop=True,
                    tile_position=(0, 0),
                )
                nc.tensor.matmul(
                    sc[:, QC:2 * QC],
                    lhsT=KT[64:128, ft, ksl],
                    rhs=QT[64:128, ft, qsl],
                    start=True, stop=True,
                    tile_position=(64, 0),
                )
                e = exps.tile([P, 2 * QC], bf16, tag="exps",
                              name=f"e{qc}_{kt}_{ft}")
                nc.scalar.activation(e, sc, EXP)
                es.append(e)
            return es

        def emit_ctx(ctxu, kt, es):
            # PSUM zero regions are whole 2KB banks: head 0's start marks
            # the bank pending-zero, so heads 1-3's first writes overwrite
            # (not accumulate) without their own start; only the final
            # head/kt closes the group.
            first, last = kt == 0, kt == NKT - 1
            for qt in range(NQT):
                for h in range(HG):
                    ft, half = divmod(h, 2)
                    nc.tensor.matmul(
                        ctxu[qt][:, h * 65:h * 65 + 65],
                        lhsT=es[ft][:, half * QC + qt * P:half * QC + (qt + 1) * P],
                        rhs=V[:, kt, h, 0:65],
                        start=first and h == 0,
                        stop=last and h == HG - 1,
                    )

        def emit_outproj_st(qc_, ctsb_, st, tail=False):
            s0 = qc_ * QC + st * P
            osb = outsb.tile([P, D], f32, tag="osb",
                             name=f"osb{qc_}_{st}")
            for jc in range(D // 512):
                ops = ps.tile([P, 512], f32, tag="sc",
                              name=f"ops{qc_}_{st}_{jc}")
                for ft in range(FT):
                    nc.tensor.matmul(
                        ops,
                        lhsT=ctsb_[:, ft, st * P:(st + 1) * P],
                        rhs=wo_sb[:, ft, jc * 512:(jc + 1) * 512],
                        start=(ft == 0),
                        stop=(ft == FT - 1),
                    )
                dst = osb[:, jc * 512:(jc + 1) * 512]
                if tail and jc == 0:
                    # ACT is idle after the last exp — keep the DVE free
                    # for the next wave's normalization (GPSIMD cannot
                    # read PSUM, so the choices are ACT and DVE)
                    nc.scalar.copy(dst, ops)
                else:
                    nc.vector.tensor_copy(out=dst, in_=ops)
                if tail:
                    nc.sync.dma_start(
                        out[s0:s0 + P, jc * 512:(jc + 1) * 512], dst
                    )
            if not tail:
                nc.sync.dma_start(out[s0:s0 + P, :], osb)

        # epilogue for one 128-q block: normalize (tensor_scalar by the
        # 1/denom column), PE-transpose back to [feat, q], pack into the
        # out-proj lhsT. ctT rides the freed "acc" slots.
        ctxus, ctsbs, ctTs = {}, {}, {}

        def alloc_epilogue(qc_):
            # Must be called BEFORE the next chunk's ctxu allocations so
            # ctT lands on the slot freed by this chunk's own qt0 — not on
            # a slot whose release depends on the next chunk completing.
            ctsbs[qc_] = ctsb_p.tile([P, FT, QC], bf16, tag="ctsb",
                                     name=f"ctsb{qc_}")
            ctTs[qc_] = ps.tile([P, FT * QC], bf16, tag="acc", bufs=4,
                                name=f"ctT{qc_}")

        ctns = {}

        def emit_norm_qt(qc_, qt, tail=False):
            cx = ctxus[qc_][qt][:, :].rearrange("p (h c) -> p h c", h=HG)
            recip = small.tile([P, HG], f32, tag="recip",
                               name=f"recip{qc_}_{qt}")
            nc.vector.reciprocal(recip, cx[:, :, 64])
            ctn = ctn_p.tile([P, HG, DK], bf16, tag="ctn",
                             name=f"ctn{qc_}_{qt}")
            for h in range(HG):
                if tail and h >= 2:
                    # tail: ACT is idle — balance the normalize across
                    # engines so the DVE chain shortens
                    nc.scalar.mul(
                        ctn[:, h, :], cx[:, h, 0:64], recip[:, h:h + 1]
                    )
                else:
                    nc.vector.tensor_scalar_mul(
                        ctn[:, h, :], cx[:, h, 0:64], recip[:, h:h + 1]
                    )
            ctns[(qc_, qt)] = ctn

        def emit_pack_qt(qc_, qt, tail=False):
            ctsb, ctT = ctsbs[qc_], ctTs[qc_]
            ctn = ctns[(qc_, qt)]
            for h in range(HG):
                ft, half = divmod(h, 2)
                nc.tensor.matmul(
                    ctT[half * 64:(half + 1) * 64,
                        ft * QC + qt * P:ft * QC + (qt + 1) * P],
                    lhsT=ctn[:, h, :],
                    rhs=ident,
                    is_transpose=True,
                    start=True, stop=True,
                )
            ctTr = ctT[:, :].rearrange("p (t q) -> p t q", t=FT)
            if tail:
                # tail: ACT is idle after the last exp — use it for the
                # pack copy so the DVE chain isn't the tail critical path
                nc.scalar.copy(
                    ctsb[:, :, qt * P:(qt + 1) * P],
                    ctTr[:, :, qt * P:(qt + 1) * P],
                )
            else:
                nc.vector.tensor_copy(
                    out=ctsb[:, :, qt * P:(qt + 1) * P],
                    in_=ctTr[:, :, qt * P:(qt + 1) * P],
                )

        def emit_epilogue_qt(qc_, qt):
            emit_norm_qt(qc_, qt)
            emit_pack_qt(qc_, qt)



        # per-kt-slot PE filler work, emitted inside the attention loop.
        # qc0 streams V chunk 0 then the K/V projections for chunks 1-3
        # (paced so chunk c lands before scores need it at kt=4c). Each
        # qc>0 runs the previous chunk's epilogue (slots 0-1) and
        # out-projection (spread one st per slot), and projects the next
        # q-chunk late in the loop.
        kv_groups = [("v", 0, st) for st in range(SCHUNK // P)]
        for c in range(1, NSC):
            kv_groups += [("k", c, ft) for ft in range(FT)]
            kv_groups += [("v", c, st) for st in range(SCHUNK // P)]
        inserts = {qc: {kt: [] for kt in range(NKT)} for qc in range(NQC)}
        for i, g in enumerate(kv_groups[:4]):   # V0: slots 0-1
            inserts[0][i // 2].append(g)
        for i, g in enumerate(kv_groups[4:]):   # 18 groups, pace 1.5/slot
            inserts[0][2 + (i * 2) // 3].append(g)
        for qc in range(NQC - 1):
            inserts[qc][11].append(("q", qc + 1, 0))
            inserts[qc][13].append(("q", qc + 1, 1))
        for qc in range(1, NQC):
            for qt in range(NQT):
                inserts[qc][qt // 2].append(("e", qc - 1, qt))
            for st in range(NQT):
                inserts[qc][3 + 2 * st].append(("o", qc - 1, st))

        for qc in range(NQC):
            if qc > 0:
                alloc_epilogue(qc - 1)
            ctxus[qc] = [
                ps.tile([P, HG * 65], f32, tag="acc", bufs=4,
                        name=f"ctxu{qc}_{qt}")
                for qt in range(NQT)
            ]
            # ctx lags scores by 2 kt so the PE's in-order stream never
            # puts ctx(kt) — which waits on both of kt's exps — ahead of
            # scores(kt+1) that feed the ACT engine.
            pend = []
            for kt in range(NKT):
                es = emit_scores_exp(qc, kt)
                pend.append((kt, es))
                for item in inserts[qc][kt]:
                    kind, a, b = item[0], item[1], item[2]
                    if kind == "k":
                        emit_kproj_g(a, b)
                    elif kind == "v":
                        emit_vproj_g(a, b)
                    elif kind == "q":
                        emit_qproj_g(a, b)
                    elif kind == "e":
                        emit_epilogue_qt(a, b)
                    else:
                        emit_outproj_st(a, ctsbs[a], b)
                if len(pend) > 2:
                    k0, e0 = pend.pop(0)
                    emit_ctx(ctxus[qc], k0, e0)
            for k0, e0 in pend:
                emit_ctx(ctxus[qc], k0, e0)

        # tail: last q-chunk's epilogue + direct-DMA out-projection,
        # pipelined per 128-q block.
        last = NQC - 1
        alloc_epilogue(last)
        for qt in range(NQT):
            emit_norm_qt(last, qt)
        for qt in range(NQT):
            emit_pack_qt(last, qt, tail=True)
            emit_outproj_st(last, ctsbs[last], qt, tail=True)

    nc.compile()
    return nc


def _get_program():
    global _PROGRAM
    if _PROGRAM is None:
        _PROGRAM = _build_program()
    return _PROGRAM


def _host_shards(q, k, v, Wq, bq, Wk, bk, Wv, bv, Wo, bo):
    """Build the 8 per-core input dicts (host-side transposes/slices)."""
    import ml_dtypes

    b16 = ml_dtypes.bfloat16
    scale = 1.0 / np.sqrt(np.float32(DK))
    qT = [np.ascontiguousarray(q[b].T).astype(b16) for b in range(B)]
    kT = [np.ascontiguousarray(k[b].T).astype(b16) for b in range(B)]
    vT = [np.ascontiguousarray(v[b].T).astype(b16) for b in range(B)]
    in_maps = []
    for c in range(NCORES):
        b, g = divmod(c, NCORES // B)
        fsl = slice(g * FC, (g + 1) * FC)
        in_maps.append({
            "qT": qT[b],
            "kT": kT[b],
            "vT": vT[b],
            "wqT": np.ascontiguousarray(Wq[fsl, :].T * scale).astype(b16),
            "wkT": np.ascontiguousarray(Wk[fsl, :].T).astype(b16),
            "wvT": np.ascontiguousarray(Wv[fsl, :].T).astype(b16),
            "woT": np.ascontiguousarray(Wo[:, fsl].T).astype(b16),
            "bq": np.ascontiguousarray(bq[fsl] * scale),
            "bk": np.ascontiguousarray(bk[fsl]),
        })
    return in_maps


def kernel(q, k, v, mask, Wq, bq, Wk, bk, Wv, bv, Wo, bo):
    q = np.asarray(q, dtype=np.float32)
    k = np.asarray(k, dtype=np.float32)
    v = np.asarray(v, dtype=np.float32)
    mask = np.asarray(mask)
    Wq = np.asarray(Wq, dtype=np.float32)
    bq = np.asarray(bq, dtype=np.float32)
    Wk = np.asarray(Wk, dtype=np.float32)
    bk = np.asarray(bk, dtype=np.float32)
    Wv = np.asarray(Wv, dtype=np.float32)
    bv = np.asarray(bv, dtype=np.float32)
    Wo = np.asarray(Wo, dtype=np.float32)
    bo = np.asarray(bo, dtype=np.float32)

    if not np.all(mask != 0):
        # Unmasked-path kernel; fall back to exact host computation if a
        # nontrivial mask ever shows up (spec fills the mask with ones).
        return _host_reference(q, k, v, mask, Wq, bq, Wk, bk, Wv, bv, Wo, bo)

    from concourse.bass_utils import run_bass_kernel_spmd

    nc = _get_program()
    in_maps = _host_shards(q, k, v, Wq, bq, Wk, bk, Wv, bv, Wo, bo)
    res = run_bass_kernel_spmd(nc, in_maps, core_ids=list(range(NCORES)))

    # host reduction: sum the 4 row-parallel Wo partials per batch,
    # then add the exact bv/bo correction (softmax rows sum to 1).
    const = bv @ Wo.T + bo
    out = np.empty((B, S, D), np.float32)
    gpb = NCORES // B
    for b in range(B):
        acc = res.results[b * gpb]["out"].astype(np.float32)
        for g in range(1, gpb):
            acc = acc + res.results[b * gpb + g]["out"]
        out[b] = acc + const[None, :]
    return out


def _host_reference(q, k, v, mask, Wq, bq, Wk, bk, Wv, bv, Wo, bo):
    def split_heads(x):
        b, s, _ = x.shape
        return x.reshape(b, s, H, DK).transpose(0, 2, 1, 3)

    query = split_heads(q @ Wq.T + bq)
    key_ = split_heads(k @ Wk.T + bk)
    value = split_heads(v @ Wv.T + bv)
    scores = np.einsum("bhqd,bhkd->bhqk", query, key_) / np.sqrt(np.float32(DK))
    scores = np.where(mask == 0, np.float32(-1e9), scores).astype(np.float32)
    scores -= scores.max(axis=-1, keepdims=True)
    e = np.exp(scores)
    attn = e / e.sum(axis=-1, keepdims=True)
    ctx = np.einsum("bhqk,bhkd->bhqd", attn, value)
    ctx = ctx.transpose(0, 2, 1, 3).reshape(q.shape[0], -1, D)
    return (ctx @ Wo.T + bo).astype(np.float32)


# revision 9
# speedup vs baseline: 1.2736x; 1.2154x over previous
"""MultiHeadAttention Trainium2 kernel (8 NeuronCores, SPMD) — v2.

Sharding: core c = (batch b=c//4, head-group g=c%4); each core owns 4 of 16
heads for one batch element. Wq/Wk/Wv split by output features (tensor
parallel on heads), Wo by input features (row parallel); the 4 partial
[S, D] outputs per batch are summed on the host.

v2 layout: ctx accumulates in [q, feat] orientation (e-tiles as the matmul
stationary, N=65 streams) so softmax denominators land as a per-partition
column and normalization is a tensor_scalar_mul — no partition broadcasts.
Normalized ctx is PE-transposed back to [feat, q] for the out-projection.
The kt loop is software-pipelined (scores/exp of kt+1 issue before ctx of
kt) so the PE never head-of-line blocks on the ACT engine's exp.
"""

import numpy as np

B, S, D = 2, 2048, 1024
H, DK = 16, 64
HG = 4                 # heads per core
FC = HG * DK           # 256 features per core
NCORES = 8
P = 128
KSUB = D // P          # 8 contraction subtiles for projections
FT = FC // P           # 2 feature tiles (= head pairs)
NKT = S // P           # 16 key-position tiles
QC = 512               # q-chunk size
NQC = S // QC          # 4
NQT = QC // P          # 4 q-subtiles per chunk
SCHUNK = 512           # s-chunk for streaming projections
NSC = S // SCHUNK      # 4

_PROGRAM = None        # cached Bass program - build once per process


def _build_program():
    from contextlib import ExitStack

    import concourse.bass as bass
    import concourse.mybir as mybir
    import concourse.tile as tile
    from concourse import bacc, masks

    f32 = mybir.dt.float32
    bf16 = mybir.dt.bfloat16
    EXP = mybir.ActivationFunctionType.Exp

    nc = bacc.Bacc("TRN2", target_bir_lowering=False, debug=False)

    qT = nc.dram_tensor("qT", [D, S], bf16, kind="ExternalInput")
    kT = nc.dram_tensor("kT", [D, S], bf16, kind="ExternalInput")
    vT = nc.dram_tensor("vT", [D, S], bf16, kind="ExternalInput")
    wqT = nc.dram_tensor("wqT", [D, FC], bf16, kind="ExternalInput")
    wkT = nc.dram_tensor("wkT", [D, FC], bf16, kind="ExternalInput")
    wvT = nc.dram_tensor("wvT", [D, FC], bf16, kind="ExternalInput")
    woT = nc.dram_tensor("woT", [FC, D], bf16, kind="ExternalInput")
    bq = nc.dram_tensor("bq", [FC], f32, kind="ExternalInput")
    bk = nc.dram_tensor("bk", [FC], f32, kind="ExternalInput")
    out = nc.dram_tensor("out", [S, D], f32, kind="ExternalOutput")

    with tile.TileContext(nc) as tc, ExitStack() as ctx, nc.allow_low_precision(
        reason="bf16 matmul operands are intentional"
    ):
        weights = ctx.enter_context(tc.tile_pool(name="weights", bufs=1))
        instream = ctx.enter_context(tc.tile_pool(name="instream", bufs=12))
        persist = ctx.enter_context(tc.tile_pool(name="persist", bufs=1))
        exps = ctx.enter_context(tc.tile_pool(name="exps", bufs=8))
        ctn_p = ctx.enter_context(tc.tile_pool(name="ctn_p", bufs=8))
        ctsb_p = ctx.enter_context(tc.tile_pool(name="ctsb_p", bufs=2))
        outsb = ctx.enter_context(tc.tile_pool(name="outsb", bufs=2))
        small = ctx.enter_context(tc.tile_pool(name="small", bufs=8))
        # PSUM: tag "sc" [128,1024]f32 = 2 banks x2 bufs (scores + qc0's
        # streamed projections + tail out-proj). tag "ctx" [128,1536]f32 =
        # 3 banks x1: ALL 16 (qt,head) ctx accumulation groups densely
        # packed (7+7+2 groups of 65 f32; none crosses a bank edge). tag
        # "ins" [128,512]f32 = 1 bank x1: steady-state insert transients
        # (out-proj tiles, next-q-chunk projection, packed ct^T) — keeping
        # them OUT of the scores rotation removes the per-insert stall.
        ps = ctx.enter_context(tc.tile_pool(name="ps", bufs=2, space="PSUM"))
        # dense ctx packing: offset of each (qt, head) 65-f32 group, plus
        # which group starts/stops each bank's accumulation (zero regions
        # are whole banks; one start marks the bank, others overwrite on
        # first touch via the pending-zero semantics)
        CTXOFF = {}
        _o = 0
        for _qt in range(NQT):
            for _h in range(HG):
                if _o % 512 + 65 > 512:
                    _o = (_o // 512 + 1) * 512
                CTXOFF[(_qt, _h)] = _o
                _o += 65
        CTX_FIRST = {(0, 0), (1, 3), (3, 2)}
        CTX_LAST = {(1, 2), (3, 1), (3, 3)}

        # ---- persistent weights (K/Q first — they gate the first scores) ----
        wk_sb = weights.tile([P, KSUB, FC], bf16, tag="wk")
        nc.sync.dma_start(wk_sb, wkT[:, :].rearrange("(o p) f -> p o f", p=P))
        wq_sb = weights.tile([P, KSUB, FC], bf16, tag="wq")
        nc.sync.dma_start(wq_sb, wqT[:, :].rearrange("(o p) f -> p o f", p=P))
        bq_sb = weights.tile([P, FT], f32, tag="bq")
        nc.sync.dma_start(bq_sb, bq[:].rearrange("(t p) -> p t", p=P))
        bk_sb = weights.tile([P, FT], f32, tag="bk")
        nc.sync.dma_start(bk_sb, bk[:].rearrange("(t p) -> p t", p=P))
        ident = weights.tile([P, P], bf16, tag="ident")
        masks.make_identity(nc, ident)
        # warm up the PE p-state while the first input DMAs are in flight
        # (independent column slices — no WAW chain, streams back-to-back)
        warm = ps.tile([P, 8 * P], bf16, tag="sc", name="warm")
        for i in range(52):
            nc.tensor.matmul(
                warm[:, (i % 8) * P:(i % 8) * P + P],
                lhsT=ident, rhs=ident, is_transpose=True,
                start=True, stop=True,
            )

        # ---- persistent activations ----
        QT = persist.tile([P, FT, S], bf16, tag="QT")   # [feat, seq]
        KT = persist.tile([P, FT, S], bf16, tag="KT")   # [feat, seq]
        V = persist.tile([P, NKT, HG, 66], bf16, tag="V")  # [seq, h, dk+ones]
        nc.vector.memset(V[:, :, :, 64:65], 1.0)

        # ---- input prefetch (all 12 chunk DMAs up front; K/V first) ----
        qTr = qT[:, :].rearrange("(o p) s -> p o s", p=P)
        kTr = kT[:, :].rearrange("(o p) s -> p o s", p=P)
        vTr = vT[:, :].rearrange("(o p) s -> p o s", p=P)
        kcs, vcs, qcs = [None] * NSC, [None] * NSC, [None] * NSC

        def dma_in(lst, src, c, nm):
            t = instream.tile([P, KSUB, SCHUNK], bf16, tag="instream",
                              name=f"{nm}{c}")
            nc.sync.dma_start(t, src[:, :, c * SCHUNK:(c + 1) * SCHUNK])
            lst[c] = t

        dma_in(kcs, kTr, 0, "kc")
        # q chunk 0 lands in two halves so its projection can start early
        qc0_t = instream.tile([P, KSUB, SCHUNK], bf16, tag="instream",
                              name="qc0")
        nc.sync.dma_start(qc0_t[:, :, 0:SCHUNK // 2],
                          qTr[:, :, 0:SCHUNK // 2])
        nc.sync.dma_start(qc0_t[:, :, SCHUNK // 2:SCHUNK],
                          qTr[:, :, SCHUNK // 2:SCHUNK])
        qcs[0] = qc0_t
        wv_sb = weights.tile([P, KSUB, FC], bf16, tag="wv")
        nc.sync.dma_start(wv_sb, wvT[:, :].rearrange("(o p) f -> p o f", p=P))
        dma_in(vcs, vTr, 0, "vc")
        wo_sb = weights.tile([P, FT, D], bf16, tag="wo")
        nc.sync.dma_start(wo_sb, woT[:, :].rearrange("(t p) j -> p t j", p=P))
        for c in range(1, NSC):
            dma_in(kcs, kTr, c, "kc")
            dma_in(vcs, vTr, c, "vc")
        for c in range(1, NSC):
            dma_in(qcs, qTr, c, "qc")

        # ---- projection group emitters: transients rotate through the
        # "sc" PSUM slots, so they can interleave with attention without
        # deadlocking against the long-lived ctx accumulators. ----
        def emit_kproj_g(c, ft, tag_="sc"):
            sl = slice(c * SCHUNK, (c + 1) * SCHUNK)
            pk = ps.tile([P, SCHUNK], f32, tag=tag_, bufs=1 if tag_ == "ins" else None,
                         name=f"pk{c}_{ft}")
            for ks in range(KSUB):
                nc.tensor.matmul(
                    pk,
                    lhsT=wk_sb[:, ks, ft * P:(ft + 1) * P],
                    rhs=kcs[c][:, ks, :],
                    start=(ks == 0),
                    stop=(ks == KSUB - 1),
                )
            nc.vector.tensor_scalar_add(KT[:, ft, sl], pk, bk_sb[:, ft:ft + 1])

        def emit_vproj_g(c, st, tag_="sc"):
            pv = ps.tile([P, FC], f32, tag=tag_, bufs=1 if tag_ == "ins" else None,
                         name=f"pv{c}_{st}")
            for ks in range(KSUB):
                nc.tensor.matmul(
                    pv,
                    lhsT=vcs[c][:, ks, st * P:(st + 1) * P],
                    rhs=wv_sb[:, ks, :],
                    start=(ks == 0),
                    stop=(ks == KSUB - 1),
                )
            kt_i = c * (SCHUNK // P) + st
            nc.vector.tensor_copy(
                out=V[:, kt_i, :, 0:DK],
                in_=pv[:, :].rearrange("p (h d) -> p h d", h=HG),
            )

        def emit_qproj_g(c, ft):
            sl = slice(c * SCHUNK, (c + 1) * SCHUNK)
            pq = ps.tile([P, SCHUNK], f32, tag="ins", bufs=1,
                         name=f"pq{c}_{ft}")
            for ks in range(KSUB):
                nc.tensor.matmul(
                    pq,
                    lhsT=wq_sb[:, ks, ft * P:(ft + 1) * P],
                    rhs=qcs[c][:, ks, :],
                    start=(ks == 0),
                    stop=(ks == KSUB - 1),
                )
            nc.vector.tensor_scalar_add(QT[:, ft, sl], pq, bq_sb[:, ft:ft + 1])

        # ---- upfront: K/Q chunk 0 only (they gate the first scores);
        # V chunk 0 and everything else streams into the attention loop.
        # Q0 is projected in 256-column halves to chase its split DMA. ----
        for ft in range(FT):
            emit_kproj_g(0, ft)
        half = SCHUNK // 2
        for hi in range(2):
            for ft in range(FT):
                pq0 = ps.tile([P, half], f32, tag="sc", name=f"pq0_{hi}_{ft}")
                for ks in range(KSUB):
                    nc.tensor.matmul(
                        pq0,
                        lhsT=wq_sb[:, ks, ft * P:(ft + 1) * P],
                        rhs=qcs[0][:, ks, hi * half:(hi + 1) * half],
                        start=(ks == 0),
                        stop=(ks == KSUB - 1),
                    )
                nc.vector.tensor_scalar_add(
                    QT[:, ft, hi * half:(hi + 1) * half], pq0,
                    bq_sb[:, ft:ft + 1],
                )

        # ---- attention, software-pipelined over kt ----
        def emit_scores_exp(qc, kt):
            qsl = slice(qc * QC, (qc + 1) * QC)
            ksl = slice(kt * P, (kt + 1) * P)
            es = []
            for ft in range(FT):
                sc = ps.tile([P, 2 * QC], f32, tag="sc",
                             name=f"sc{qc}_{kt}_{ft}")
                nc.tensor.matmul(
                    sc[:, 0:QC],
                    lhsT=KT[0:64, ft, ksl],
                    rhs=QT[0:64, ft, qsl],
                    start=True, stop=True,
                    tile_position=(0, 0),
                )
                nc.tensor.matmul(
                    sc[:, QC:2 * QC],
                    lhsT=KT[64:128, ft, ksl],
                    rhs=QT[64:128, ft, qsl],
                    start=True, stop=True,
                    tile_position=(64, 0),
                )
                e = exps.tile([P, 2 * QC], bf16, tag="exps",
                              name=f"e{qc}_{kt}_{ft}")
                nc.scalar.activation(e, sc, EXP)
                es.append(e)
            return es

        def emit_ctx(ctxp, kt, es):
            # one densely packed accumulator tile: per bank, exactly one
            # group carries start (marks the bank pending-zero; the other
            # groups' first writes overwrite) and one carries stop.
            first, last = kt == 0, kt == NKT - 1
            for qt in range(NQT):
                for h in range(HG):
                    ft, half = divmod(h, 2)
                    o = CTXOFF[(qt, h)]
                    nc.tensor.matmul(
                        ctxp[:, o:o + 65],
                        lhsT=es[ft][:, half * QC + qt * P:half * QC + (qt + 1) * P],
                        rhs=V[:, kt, h, 0:65],
                        start=first and (qt, h) in CTX_FIRST,
                        stop=last and (qt, h) in CTX_LAST,
                    )

        osbs = {}

        def emit_outproj_jc(qc_, ctsb_, st, jc):
            # steady-state: one jc half per insert, transient on the
            # dedicated "ins" bank — never perturbs the scores rotation
            s0 = qc_ * QC + st * P
            if jc == 0:
                osbs[(qc_, st)] = outsb.tile([P, D], f32, tag="osb",
                                             name=f"osb{qc_}_{st}")
            osb = osbs[(qc_, st)]
            ops = ps.tile([P, 512], f32, tag="ins", bufs=1,
                          name=f"opsi{qc_}_{st}_{jc}")
            for ft in range(FT):
                nc.tensor.matmul(
                    ops,
                    lhsT=ctsb_[:, ft, st * P:(st + 1) * P],
                    rhs=wo_sb[:, ft, jc * 512:(jc + 1) * 512],
                    start=(ft == 0),
                    stop=(ft == FT - 1),
                )
            nc.vector.tensor_copy(
                out=osb[:, jc * 512:(jc + 1) * 512], in_=ops
            )
            if jc == 1:
                nc.sync.dma_start(out[s0:s0 + P, :], osb)

        def emit_outproj_st(qc_, ctsb_, st, tail=False):
            s0 = qc_ * QC + st * P
            osb = outsb.tile([P, D], f32, tag="osb",
                             name=f"osb{qc_}_{st}")
            for jc in range(D // 512):
                ops = ps.tile([P, 512], f32, tag="sc",
                              name=f"ops{qc_}_{st}_{jc}")
                for ft in range(FT):
                    nc.tensor.matmul(
                        ops,
                        lhsT=ctsb_[:, ft, st * P:(st + 1) * P],
                        rhs=wo_sb[:, ft, jc * 512:(jc + 1) * 512],
                        start=(ft == 0),
                        stop=(ft == FT - 1),
                    )
                dst = osb[:, jc * 512:(jc + 1) * 512]
                if tail and jc == 0:
                    # ACT is idle after the last exp — keep the DVE free
                    # for the next wave's normalization (GPSIMD cannot
                    # read PSUM, so the choices are ACT and DVE)
                    nc.scalar.copy(dst, ops)
                else:
                    nc.vector.tensor_copy(out=dst, in_=ops)
                if tail:
                    nc.sync.dma_start(
                        out[s0:s0 + P, jc * 512:(jc + 1) * 512], dst
                    )
            if not tail:
                nc.sync.dma_start(out[s0:s0 + P, :], osb)

        # epilogue for one 128-q block: normalize (tensor_scalar by the
        # 1/denom column), PE-transpose back to [feat, q], pack into the
        # out-proj lhsT. ctT rides the freed "acc" slots.
        ctxus, ctsbs, ctTs = {}, {}, {}

        def alloc_epilogue(qc_):
            ctsbs[qc_] = ctsb_p.tile([P, FT, QC], bf16, tag="ctsb",
                                     name=f"ctsb{qc_}")
            ctTs[qc_] = ps.tile([P, FT * QC], bf16, tag="ins", bufs=1,
                                name=f"ctT{qc_}")

        ctns = {}

        def emit_norm_qt(qc_, qt, tail=False):
            ctxp = ctxus[qc_]
            recip = small.tile([P, HG], f32, tag="recip",
                               name=f"recip{qc_}_{qt}")
            for h in range(HG):
                o = CTXOFF[(qt, h)]
                nc.vector.reciprocal(recip[:, h:h + 1], ctxp[:, o + 64:o + 65])
            ctn = ctn_p.tile([P, HG, DK], bf16, tag="ctn",
                             name=f"ctn{qc_}_{qt}")
            for h in range(HG):
                o = CTXOFF[(qt, h)]
                nc.vector.tensor_scalar_mul(
                    ctn[:, h, :], ctxp[:, o:o + 64], recip[:, h:h + 1]
                )
            ctns[(qc_, qt)] = ctn

        def emit_pack_qt(qc_, qt, tail=False):
            ctsb, ctT = ctsbs[qc_], ctTs[qc_]
            ctn = ctns[(qc_, qt)]
            for h in range(HG):
                ft, half = divmod(h, 2)
                nc.tensor.matmul(
                    ctT[half * 64:(half + 1) * 64,
                        ft * QC + qt * P:ft * QC + (qt + 1) * P],
                    lhsT=ctn[:, h, :],
                    rhs=ident,
                    is_transpose=True,
                    start=True, stop=True,
                )
            ctTr = ctT[:, :].rearrange("p (t q) -> p t q", t=FT)
            if tail:
                # tail: ACT is idle after the last exp — use it for the
                # pack copy so the DVE chain isn't the tail critical path
                nc.scalar.copy(
                    ctsb[:, :, qt * P:(qt + 1) * P],
                    ctTr[:, :, qt * P:(qt + 1) * P],
                )
            else:
                nc.vector.tensor_copy(
                    out=ctsb[:, :, qt * P:(qt + 1) * P],
                    in_=ctTr[:, :, qt * P:(qt + 1) * P],
                )

        def emit_epilogue_qt(qc_, qt):
            emit_norm_qt(qc_, qt)
            emit_pack_qt(qc_, qt)



        # per-kt-slot PE filler work, emitted inside the attention loop.
        # qc0 streams V chunk 0 then the K/V projections for chunks 1-3
        # (paced so chunk c lands before scores need it at kt=4c). Each
        # qc>0 runs the previous chunk's epilogue (slots 0-1) and
        # out-projection (spread one st per slot), and projects the next
        # q-chunk late in the loop.
        inserts = {qc: {kt: [] for kt in range(NKT)} for qc in range(NQC)}
        # qc0: K-chunk groups ride the "ins" bank (free until slot 12, one
        # group per slot so each consumer clears before the next alloc);
        # V groups pace through the scores rotation, ahead of their ctx
        # deadlines at kt=4c+st.
        for i, (c, ft) in enumerate(
            (c, ft) for c in range(1, NSC) for ft in range(FT)
        ):
            inserts[0][1 + 3 * (i // 2) + (i % 2)].append(("ki", c, ft))
        for st in range(SCHUNK // P):           # V0: slots 0-1
            inserts[0][st // 2].append(("v", 0, st))
        for c in range(1, 3):                   # V1-2: one per slot (sc)
            for st in range(SCHUNK // P):
                inserts[0][4 * (c - 1) + st + 2].append(("v", c, st))
        for st in range(SCHUNK // P):           # V3: ins slots 9-12
            inserts[0][9 + st].append(("vi", 3, st))
        inserts[0][13].append(("q", 1, 0))
        inserts[0][14].append(("q", 1, 1))
        for qc in range(1, NQC - 1):
            inserts[qc][12].append(("q", qc + 1, 0))
            inserts[qc][13].append(("q", qc + 1, 1))
        for qc in range(1, NQC):
            for qt in range(NQT):
                inserts[qc][qt // 2].append(("e", qc - 1, qt))
            for st in range(NQT):
                for jc in range(2):
                    inserts[qc][3 + 2 * st + jc].append(
                        ("oj", qc - 1, st, jc)
                    )

        for qc in range(NQC):
            if qc > 0:
                alloc_epilogue(qc - 1)
            ctxus[qc] = ps.tile([P, 3 * 512], f32, tag="ctx", bufs=1,
                                name=f"ctxp{qc}")
            # ctx lags scores by 2 kt so the PE's in-order stream never
            # puts ctx(kt) — which waits on both of kt's exps — ahead of
            # scores(kt+1) that feed the ACT engine.
            pend = []
            for kt in range(NKT):
                es = emit_scores_exp(qc, kt)
                pend.append((kt, es))
                for item in inserts[qc][kt]:
                    kind, a, b = item[0], item[1], item[2]
                    if kind == "k":
                        emit_kproj_g(a, b)
                    elif kind == "ki":
                        emit_kproj_g(a, b, tag_="ins")
                    elif kind == "vi":
                        emit_vproj_g(a, b, tag_="ins")
                    elif kind == "v":
                        emit_vproj_g(a, b)
                    elif kind == "q":
                        emit_qproj_g(a, b)
                    elif kind == "e":
                        emit_epilogue_qt(a, b)
                    else:
                        emit_outproj_jc(a, ctsbs[a], b, item[3])
                if len(pend) > 2:
                    k0, e0 = pend.pop(0)
                    emit_ctx(ctxus[qc], k0, e0)
            for k0, e0 in pend:
                emit_ctx(ctxus[qc], k0, e0)

        # tail: last q-chunk's epilogue + direct-DMA out-projection,
        # pipelined per 128-q block.
        last = NQC - 1
        alloc_epilogue(last)
        for qt in range(NQT):
            emit_norm_qt(last, qt)
        for qt in range(NQT):
            emit_pack_qt(last, qt, tail=True)
            emit_outproj_st(last, ctsbs[last], qt, tail=True)

    nc.compile()
    return nc


def _get_program():
    global _PROGRAM
    if _PROGRAM is None:
        _PROGRAM = _build_program()
    return _PROGRAM


def _host_shards(q, k, v, Wq, bq, Wk, bk, Wv, bv, Wo, bo):
    """Build the 8 per-core input dicts (host-side transposes/slices)."""
    import ml_dtypes

    b16 = ml_dtypes.bfloat16
    scale = 1.0 / np.sqrt(np.float32(DK))
    qT = [np.ascontiguousarray(q[b].T).astype(b16) for b in range(B)]
    kT = [np.ascontiguousarray(k[b].T).astype(b16) for b in range(B)]
    vT = [np.ascontiguousarray(v[b].T).astype(b16) for b in range(B)]
    in_maps = []
    for c in range(NCORES):
        b, g = divmod(c, NCORES // B)
        fsl = slice(g * FC, (g + 1) * FC)
        in_maps.append({
            "qT": qT[b],
            "kT": kT[b],
            "vT": vT[b],
            "wqT": np.ascontiguousarray(Wq[fsl, :].T * scale).astype(b16),
            "wkT": np.ascontiguousarray(Wk[fsl, :].T).astype(b16),
            "wvT": np.ascontiguousarray(Wv[fsl, :].T).astype(b16),
            "woT": np.ascontiguousarray(Wo[:, fsl].T).astype(b16),
            "bq": np.ascontiguousarray(bq[fsl] * scale),
            "bk": np.ascontiguousarray(bk[fsl]),
        })
    return in_maps


def kernel(q, k, v, mask, Wq, bq, Wk, bk, Wv, bv, Wo, bo):
    q = np.asarray(q, dtype=np.float32)
    k = np.asarray(k, dtype=np.float32)
    v = np.asarray(v, dtype=np.float32)
    mask = np.asarray(mask)
    Wq = np.asarray(Wq, dtype=np.float32)
    bq = np.asarray(bq, dtype=np.float32)
    Wk = np.asarray(Wk, dtype=np.float32)
    bk = np.asarray(bk, dtype=np.float32)
    Wv = np.asarray(Wv, dtype=np.float32)
    bv = np.asarray(bv, dtype=np.float32)
    Wo = np.asarray(Wo, dtype=np.float32)
    bo = np.asarray(bo, dtype=np.float32)

    if not np.all(mask != 0):
        # Unmasked-path kernel; fall back to exact host computation if a
        # nontrivial mask ever shows up (spec fills the mask with ones).
        return _host_reference(q, k, v, mask, Wq, bq, Wk, bk, Wv, bv, Wo, bo)

    from concourse.bass_utils import run_bass_kernel_spmd

    nc = _get_program()
    in_maps = _host_shards(q, k, v, Wq, bq, Wk, bk, Wv, bv, Wo, bo)
    res = run_bass_kernel_spmd(nc, in_maps, core_ids=list(range(NCORES)))

    # host reduction: sum the 4 row-parallel Wo partials per batch,
    # then add the exact bv/bo correction (softmax rows sum to 1).
    const = bv @ Wo.T + bo
    out = np.empty((B, S, D), np.float32)
    gpb = NCORES // B
    for b in range(B):
        acc = res.results[b * gpb]["out"].astype(np.float32)
        for g in range(1, gpb):
            acc = acc + res.results[b * gpb + g]["out"]
        out[b] = acc + const[None, :]
    return out


def _host_reference(q, k, v, mask, Wq, bq, Wk, bk, Wv, bv, Wo, bo):
    def split_heads(x):
        b, s, _ = x.shape
        return x.reshape(b, s, H, DK).transpose(0, 2, 1, 3)

    query = split_heads(q @ Wq.T + bq)
    key_ = split_heads(k @ Wk.T + bk)
    value = split_heads(v @ Wv.T + bv)
    scores = np.einsum("bhqd,bhkd->bhqk", query, key_) / np.sqrt(np.float32(DK))
    scores = np.where(mask == 0, np.float32(-1e9), scores).astype(np.float32)
    scores -= scores.max(axis=-1, keepdims=True)
    e = np.exp(scores)
    attn = e / e.sum(axis=-1, keepdims=True)
    ctx = np.einsum("bhqk,bhkd->bhqd", attn, value)
    ctx = ctx.transpose(0, 2, 1, 3).reshape(q.shape[0], -1, D)
    return (ctx @ Wo.T + bo).astype(np.float32)
